# revision 6
# baseline (speedup 1.0000x reference)
"""Distributed Bjorck-Bowie orthonormalization of a 4096x4096 fp32 matrix
on 8 Trainium2 NeuronCores.

Algorithm (reference):
    s = 1/sqrt(max col abs-sum * max row abs-sum)
    w = W * s
    12x:  G = w^T w ;  w = 1.5 w - 0.5 w G

Distribution: column-sharded. Core i owns C = w[:, 512i:512(i+1)] (fp32
master + matmul-dtype copy in SBUF). Each iteration both w and w^T are
available in DRAM as tile-blocked AllGather outputs:
  wst  = w   as [32 col-tile][32 row-tile][128][128]   (natural)
  wstT = w^T as [32 row-tile][32 col-tile][128][128]   (transposed)
Per core, per iteration:
  phase A: wtwn = -0.5 * G[:, own] : out[r, a] = sum_k w[k,r] C[k,a]
           stationary = wst panels (col-slice of w), moving = c_mm tiles
  phase B: psU = -0.5 * (w G)[:, own] : out[m, a] = sum_k wT[k,m] wtwn[k,a]
           stationary = wstT panels, moving = wtwn tiles
  epilogue (fused, per row-tile): c_master = 1.5*c_master + psU
  own-block transposes feed the next AllGather pair.

Matmul dtype configurable: bfloat16 (fp32 masters, ~1.4e-3 rel) or
float32r (13-bit mantissa, ~2e-4 rel, 2x HBM/AG bytes).
"""

import os

import numpy as np

import concourse.mybir as mybir
import concourse.tile as tile
from concourse import bacc
from concourse.bass import ts
from concourse.bass_utils import run_bass_kernel_spmd
from concourse.masks import make_identity

N_CORES = 8
D = 4096                # matrix dim
B = D // N_CORES        # 512 columns per core
P = 128                 # partitions
NT = D // P             # 32 tiles of 128
NBT = B // P            # 4 own-col tiles
ITERS = int(os.environ.get("BB_ITERS", "12"))
MM_DTYPE = os.environ.get("BB_MM_DTYPE", "bfloat16")  # bfloat16 | float32r

f32 = mybir.dt.float32


def _build():
    mmdt = getattr(mybir.dt, MM_DTYPE)
    use_master = MM_DTYPE == "bfloat16"

    nc = bacc.Bacc(
        "TRN2",
        target_bir_lowering=False,
        debug=False,
        num_devices=N_CORES,
    )
    wblk = nc.dram_tensor("wblk", [D, B], f32, kind="ExternalInput").ap()
    out = nc.dram_tensor("out", [D, B], f32, kind="ExternalOutput").ap()

    rg = [list(range(N_CORES))]

    with tile.TileContext(nc) as tc:
        with (
            tc.tile_pool(name="big", bufs=1) as big,
            tc.tile_pool(name="panels", bufs=4) as panels,
            tc.tile_pool(name="work", bufs=3) as work,
            tc.tile_pool(name="const", bufs=1) as const,
            tc.tile_pool(name="psmm", bufs=4, space="PSUM") as psmm,
            tc.tile_pool(name="pssmall", bufs=3, space="PSUM") as pssmall,
            tc.tile_pool(name="dram", bufs=1, space="DRAM") as dram,
        ):
            # ---- persistent state ----
            if use_master:
                c_master = big.tile([P, NT, B], f32)
            c_mm = big.tile([P, NT, B], mmdt)
            wtwn = big.tile([P, NT, B], mmdt)

            ident_mm = const.tile([P, P], mmdt)
            make_identity(nc, ident_mm)
            ident_f32 = const.tile([P, P], f32)
            make_identity(nc, ident_f32)
            ones_col = const.tile([P, 1], mmdt)
            nc.vector.memset(ones_col[:], 1.0)
            ones_row = const.tile([1, P], f32)
            nc.vector.memset(ones_row[:], 1.0)

            # AllGather buffers: tile-blocked [32 g][32 t][128][128].
            # wst slot (g=col-tile, t=row-tile); wstT slot (g=row-tile of w^T
            # = col-tile of w, t=col-tile of w^T = row-tile of w).
            wst = [
                dram.tile([NT * NT * P, P], mmdt, addr_space="Shared",
                          name=f"wst{j}")
                for j in range(ITERS)
            ]
            wstT = [
                dram.tile([NT * NT * P, P], mmdt, addr_space="Shared",
                          name=f"wstT{j}")
                for j in range(ITERS)
            ]
            ag_in = [
                dram.tile([NBT * NT * P, P], mmdt, name=f"ag_in{j}")
                for j in range(ITERS)
            ]
            agT_in = [
                dram.tile([NBT * NT * P, P], mmdt, name=f"agT_in{j}")
                for j in range(ITERS)
            ]

            def emit_gathers(j):
                """DMA own block (natural + transposed, tiled) and AllGather."""
                # natural: slot (nt, kt) <- c_mm[:, kt, nt*128:+128]
                for nt in range(NBT):
                    agr = ag_in[j][nt * NT * P: (nt + 1) * NT * P, :].rearrange(
                        "(kt p) c -> p kt c", kt=NT, p=P
                    )
                    nc.sync.dma_start(
                        out=agr, in_=c_mm[:, :, ts(nt, P)]
                    )
                nc.gpsimd.collective_compute(
                    "AllGather", mybir.AluOpType.bypass, replica_groups=rg,
                    ins=[ag_in[j].opt()], outs=[wst[j].opt()],
                )
                nc.gpsimd.collective_compute(
                    "AllGather", mybir.AluOpType.bypass, replica_groups=rg,
                    ins=[agT_in[j].opt()], outs=[wstT[j].opt()],
                )

            def emit_transposes(j, mt_range):
                """PE-transpose own-block tiles into agT_in[j] (slot (qt, mt))."""
                for mt in mt_range:
                    for qt in range(NBT):
                        pstm = pssmall.tile(
                            [P, 512], mmdt, tag="small", name="pstm"
                        )
                        nc.tensor.transpose(
                            pstm[:, 0:P], c_mm[:, mt, ts(qt, P)], ident_mm[:]
                        )
                        stg = work.tile([P, P], mmdt, name="stg")
                        nc.any.tensor_copy(stg[:], pstm[:, 0:P])
                        nc.sync.dma_start(
                            out=agT_in[j][(qt * NT + mt) * P: (qt * NT + mt + 1) * P, :],
                            in_=stg[:],
                        )

            # ================= preamble: load + scale =================
            if use_master:
                stage = c_master
            else:
                stage = big.tile([P, NT, B], f32, name="stage")
            for kt in range(NT):
                nc.sync.dma_start(out=stage[:, kt, :], in_=wblk[ts(kt, P), :])

            # row-sums of |.| over own cols -> rs[128, NT]
            rs = const.tile([P, NT], f32)
            for kt in range(NT):
                nc.vector.tensor_reduce(
                    rs[:, kt: kt + 1],
                    stage[:, kt, :],
                    axis=mybir.AxisListType.X,
                    op=mybir.AluOpType.add,
                    apply_absolute_value=True,
                )
            # col-sums via ones-matmul over |tiles| -> [1, B]
            ps_cs = pssmall.tile([P, 512], f32, tag="small", name="ps_cs")
            for kt in range(NT):
                babs = work.tile([P, B], mmdt, name="babs")
                nc.scalar.activation(
                    babs[:], stage[:, kt, :], mybir.ActivationFunctionType.Abs
                )
                nc.tensor.matmul(
                    ps_cs[0:1, 0:B],
                    ones_col[:],
                    babs[:],
                    start=(kt == 0),
                    stop=(kt == NT - 1),
                )
            cs_sb = const.tile([1, B], f32)
            nc.scalar.copy(cs_sb[:], ps_cs[0:1, 0:B])
            cmax_l = const.tile([1, 1], f32)
            nc.vector.tensor_reduce(
                cmax_l[:], cs_sb[:], axis=mybir.AxisListType.X,
                op=mybir.AluOpType.max,
            )

            # AllReduce row-sums (add) + col-max (max)
            rs_d = dram.tile([P, NT], f32)
            rs_do = dram.tile([P, NT], f32, addr_space="Shared")
            cm_d = dram.tile([1, 1], f32)
            cm_do = dram.tile([1, 1], f32, addr_space="Shared")
            nc.sync.dma_start(out=rs_d[:], in_=rs[:])
            nc.sync.dma_start(out=cm_d[:], in_=cmax_l[:])
            nc.gpsimd.collective_compute(
                "AllReduce", mybir.AluOpType.add, replica_groups=rg,
                ins=[rs_d.opt()], outs=[rs_do.opt()],
            )
            nc.gpsimd.collective_compute(
                "AllReduce", mybir.AluOpType.max, replica_groups=rg,
                ins=[cm_d.opt()], outs=[cm_do.opt()],
            )
            rs_full = const.tile([P, NT], f32)
            cmax = const.tile([1, 1], f32)
            nc.sync.dma_start(out=rs_full[:], in_=rs_do[:])
            nc.sync.dma_start(out=cmax[:], in_=cm_do[:])

            rvec = const.tile([P, 1], f32)
            nc.vector.tensor_reduce(
                rvec[:], rs_full[:], axis=mybir.AxisListType.X,
                op=mybir.AluOpType.max,
            )
            ps_t = pssmall.tile([P, 512], f32, tag="small", name="ps_t")
            nc.tensor.transpose(ps_t[0:1, 0:P], rvec[:], ident_f32[:])
            rvec_t = const.tile([1, P], f32)
            nc.scalar.copy(rvec_t[:], ps_t[0:1, 0:P])
            rmax = const.tile([1, 1], f32)
            nc.vector.tensor_reduce(
                rmax[:], rvec_t[:], axis=mybir.AxisListType.X,
                op=mybir.AluOpType.max,
            )

            # s = 1/sqrt(rmax*cmax); broadcast to [128,1]
            prod = const.tile([1, 1], f32)
            nc.vector.tensor_tensor(
                out=prod[:], in0=rmax[:], in1=cmax[:], op=mybir.AluOpType.mult
            )
            sq = const.tile([1, 1], f32)
            nc.scalar.sqrt(sq[:], prod[:])
            sval = const.tile([1, 1], f32)
            nc.vector.reciprocal(sval[:], sq[:])
            ps_b = pssmall.tile([P, 512], f32, tag="small", name="ps_b")
            nc.tensor.matmul(
                ps_b[0:P, 0:1], ones_row[:], sval[:], start=True, stop=True
            )
            svec = const.tile([P, 1], f32)
            nc.scalar.copy(svec[:], ps_b[0:P, 0:1])

            for kt in range(NT):
                if use_master:
                    nc.scalar.activation(
                        c_master[:, kt, :], c_master[:, kt, :],
                        mybir.ActivationFunctionType.Copy, scale=svec[:],
                    )
                    nc.vector.tensor_copy(c_mm[:, kt, :], c_master[:, kt, :])
                else:
                    nc.scalar.activation(
                        c_mm[:, kt, :], stage[:, kt, :],
                        mybir.ActivationFunctionType.Copy, scale=svec[:],
                    )

            emit_transposes(0, range(NT))
            emit_gathers(0)

            # ================= iterations =================
            for it in range(ITERS):
                last = it == ITERS - 1
                wv = wst[it].rearrange("(g t p) c -> g p t c", g=NT, t=NT, p=P)
                wTv = wstT[it].rearrange("(g t p) c -> t p g c", g=NT, t=NT, p=P)

                # phase A: wtwn[:, rt, :] = -0.5 * G[rt-block, own-cols]
                for rt in range(NT):
                    pa = panels.tile([P, NT, P], mmdt, tag="panel", name="pa")
                    nc.sync.dma_start(out=pa[:], in_=wv[rt])
                    psg = psmm.tile([P, B], f32, tag="mm", name="psg")
                    for kt in range(NT):
                        nc.tensor.matmul(
                            psg[:],
                            pa[:, kt, :],
                            c_mm[:, kt, :],
                            start=(kt == 0),
                            stop=(kt == NT - 1),
                        )
                    nc.scalar.activation(
                        wtwn[:, rt, :], psg[:],
                        mybir.ActivationFunctionType.Copy, scale=-0.5,
                    )

                # phase B + fused epilogue per row-tile mt
                for mt in range(NT):
                    pt = panels.tile([P, NT, P], mmdt, tag="panel", name="pt")
                    nc.sync.dma_start(out=pt[:], in_=wTv[mt])
                    psu = psmm.tile([P, B], f32, tag="mm", name="psu")
                    for g in range(NT):
                        nc.tensor.matmul(
                            psu[:],
                            pt[:, g, :],
                            wtwn[:, g, :],
                            start=(g == 0),
                            stop=(g == NT - 1),
                        )
                    if use_master:
                        nc.vector.scalar_tensor_tensor(
                            out=c_master[:, mt, :],
                            in0=c_master[:, mt, :],
                            scalar=1.5,
                            in1=psu[:],
                            op0=mybir.AluOpType.mult,
                            op1=mybir.AluOpType.add,
                        )
                        nc.vector.tensor_copy(c_mm[:, mt, :], c_master[:, mt, :])
                    else:
                        nc.vector.scalar_tensor_tensor(
                            out=c_mm[:, mt, :],
                            in0=c_mm[:, mt, :],
                            scalar=1.5,
                            in1=psu[:],
                            op0=mybir.AluOpType.mult,
                            op1=mybir.AluOpType.add,
                        )
                    if not last:
                        emit_transposes(it + 1, [mt])

                if not last:
                    emit_gathers(it + 1)

            # ================= output =================
            outr = out.rearrange("(kt p) n -> p kt n", p=P)
            if use_master:
                nc.sync.dma_start(out=outr, in_=c_master[:, :, :])
            else:
                nc.sync.dma_start(out=outr, in_=c_mm.bitcast(f32)[:, :, :])

    nc.compile()
    return nc


_NC_CACHE = {}


def _get_nc():
    key = (ITERS, MM_DTYPE)
    if key not in _NC_CACHE:
        _NC_CACHE[key] = _build()
    return _NC_CACHE[key]


def kernel(weight: np.ndarray, **kwargs) -> np.ndarray:
    assert weight.shape == (D, D) and weight.dtype == np.float32
    nc = _get_nc()
    in_maps = [
        {"wblk": np.ascontiguousarray(weight[:, c * B: (c + 1) * B])}
        for c in range(N_CORES)
    ]
    res = run_bass_kernel_spmd(
        nc, in_maps, core_ids=list(range(N_CORES)),
        trace=bool(int(os.environ.get("BB_TRACE", "0"))),
    )
    full = np.concatenate(
        [res.results[c]["out"] for c in range(N_CORES)], axis=1
    )
    if kwargs.get("return_res"):
        return full, res
    return full


# revision 8
# speedup vs baseline: 1.1785x; 1.1785x over previous
"""Distributed Bjorck-Bowie orthonormalization of a 4096x4096 fp32 matrix
on 8 Trainium2 NeuronCores.

Algorithm (reference):
    s = 1/sqrt(max col abs-sum * max row abs-sum)
    w = W * s
    12x:  G = w^T w ;  w = 1.5 w - 0.5 w G

Distribution: column-sharded. Core i owns C = w[:, 512i:512(i+1)] (fp32
master + matmul-dtype copy in SBUF). Both w and w^T are regathered every
iteration in partition-major tile layouts:
  wst (4 chunks, one per own-col tile nt): chunk = AllGather of
      ag_in[nt*128:(nt+1)*128] where ag_in row (nt*128+p) holds
      [kt, c]-contiguous 8KB spans -> A-panels stream at full DMA width.
  wstT: single AllGather of agT_in, row (mt*512 + p*4 + qt), giving
      1KB-contiguous B-panel lines.
Per core, per iteration:
  phase A: wtwn = -0.5 * G[:, own]; out[r, a] = sum_k w[k,r] C[k,a]
           stationary = A-panels (8 per chunk), moving = c_mm tiles
  phase B: psU = -0.5 * (w G)[:, own]; stationary = B-panels, moving = wtwn
  epilogue (fused): c_master = 1.5*c_master + psU; cast c_mm; PE-transpose
      own tiles into the next agT_in.
AG(wst) is chunked so phase A starts ~38us after the epilogue; AG(wstT)
hides under phase A.

Matmul dtype: bfloat16 (fp32 masters, ~1.4e-3 rel) or float32r
(13-bit mantissa, ~2e-4 rel, 2x bytes).
"""

import os

import numpy as np

import concourse.mybir as mybir
import concourse.tile as tile
from concourse import bacc
from concourse.bass import ts
from concourse.bass_utils import run_bass_kernel_spmd
from concourse.masks import make_identity

N_CORES = 8
D = 4096
B = D // N_CORES        # 512
P = 128
NT = D // P             # 32
NBT = B // P            # 4
ITERS = int(os.environ.get("BB_ITERS", "12"))
MM_DTYPE = os.environ.get("BB_MM_DTYPE", "bfloat16")

f32 = mybir.dt.float32


def _build():
    mmdt = getattr(mybir.dt, MM_DTYPE)
    use_master = MM_DTYPE == "bfloat16"

    nc = bacc.Bacc(
        "TRN2",
        target_bir_lowering=False,
        debug=False,
        num_devices=N_CORES,
    )
    wblk = nc.dram_tensor("wblk", [D, B], f32, kind="ExternalInput").ap()
    out = nc.dram_tensor("out", [D, B], f32, kind="ExternalOutput").ap()

    rg = [list(range(N_CORES))]

    with tile.TileContext(nc) as tc:
        with (
            tc.tile_pool(name="big", bufs=1) as big,
            tc.tile_pool(name="panels", bufs=4) as panels,
            tc.tile_pool(name="work", bufs=3) as work,
            tc.tile_pool(name="const", bufs=1) as const,
            tc.tile_pool(name="psmm", bufs=4, space="PSUM") as psmm,
            tc.tile_pool(name="pssmall", bufs=3, space="PSUM") as pssmall,
            tc.tile_pool(name="dram", bufs=1, space="DRAM") as dram,
        ):
            # ---- persistent state ----
            if use_master:
                c_master = big.tile([P, NT, B], f32)
            c_mm = big.tile([P, NT, B], mmdt)
            wtwn = big.tile([P, NT, B], mmdt)

            ident_mm = const.tile([P, P], mmdt)
            make_identity(nc, ident_mm)
            ident_f32 = const.tile([P, P], f32)
            make_identity(nc, ident_f32)
            ones_col = const.tile([P, 1], mmdt)
            nc.vector.memset(ones_col[:], 1.0)
            ones_row = const.tile([1, P], f32)
            nc.vector.memset(ones_row[:], 1.0)

            # AllGather buffers.
            # ag_in[j]: [512, 4096]; row nt*128+p holds (kt,c) spans of
            #   c_mm[p, :, nt*128:+128]  (8KB contiguous per row)
            # wstc[j][nt]: AG out [8*128, 4096] (rank-stacked chunk)
            # agT_in[j]: [16384, 128]; row mt*512 + p*4 + qt = transposed
            #   tile lines; wstT[j]: AG out [8*16384, 128]
            ag_in = [
                dram.tile([NBT * P, NT * P], mmdt, name=f"ag_in{j}")
                for j in range(ITERS)
            ]
            wstc = [
                [
                    dram.tile([N_CORES * P, NT * P], mmdt,
                              addr_space="Shared", name=f"wstc{j}_{nt}")
                    for nt in range(NBT)
                ]
                for j in range(ITERS)
            ]
            agT_in = [
                dram.tile([NT * NBT * P, P], mmdt, name=f"agT_in{j}")
                for j in range(ITERS)
            ]
            wstT = [
                dram.tile([N_CORES * NT * NBT * P, P], mmdt,
                          addr_space="Shared", name=f"wstT{j}")
                for j in range(ITERS)
            ]

            def emit_gathers(j):
                for nt in range(NBT):
                    agr = ag_in[j][nt * P: (nt + 1) * P, :].rearrange(
                        "p (kt c) -> p kt c", kt=NT, c=P
                    )
                    nc.scalar.dma_start(out=agr, in_=c_mm[:, :, ts(nt, P)])
                for nt in range(NBT):
                    nc.gpsimd.collective_compute(
                        "AllGather", mybir.AluOpType.bypass, replica_groups=rg,
                        ins=[ag_in[j][nt * P: (nt + 1) * P, :].opt()],
                        outs=[wstc[j][nt].opt()],
                    )
                nc.gpsimd.collective_compute(
                    "AllGather", mybir.AluOpType.bypass, replica_groups=rg,
                    ins=[agT_in[j].opt()], outs=[wstT[j].opt()],
                )

            def emit_transposes(j, mt_range):
                """Own-block transposed tiles -> agT_in[j] rows mt*512+p*4+qt."""
                for mt in mt_range:
                    pstm = pssmall.tile([P, 512], mmdt, tag="small", name="pstm")
                    for qt in range(NBT):
                        nc.tensor.transpose(
                            pstm[:, ts(qt, P)], c_mm[:, mt, ts(qt, P)],
                            ident_mm[:],
                        )
                    stg = work.tile([P, NBT * P], mmdt, name="stg")
                    nc.scalar.copy(stg[:], pstm[:])
                    o = agT_in[j][mt * NBT * P: (mt + 1) * NBT * P, :]
                    nc.gpsimd.dma_start(
                        out=o.rearrange("(p qt) c -> p qt c", p=P, qt=NBT),
                        in_=stg.rearrange("p (qt c) -> p qt c", qt=NBT),
                    )

            # ================= preamble: load + scale =================
            if use_master:
                stage = c_master
            else:
                stage = big.tile([P, NT, B], f32, name="stage")
            for kt in range(NT):
                nc.sync.dma_start(out=stage[:, kt, :], in_=wblk[ts(kt, P), :])

            rs = const.tile([P, NT], f32)
            for kt in range(NT):
                nc.vector.tensor_reduce(
                    rs[:, kt: kt + 1],
                    stage[:, kt, :],
                    axis=mybir.AxisListType.X,
                    op=mybir.AluOpType.add,
                    apply_absolute_value=True,
                )
            ps_cs = pssmall.tile([P, 512], f32, tag="small", name="ps_cs")
            for kt in range(NT):
                babs = work.tile([P, B], mmdt, name="babs")
                nc.scalar.activation(
                    babs[:], stage[:, kt, :], mybir.ActivationFunctionType.Abs
                )
                nc.tensor.matmul(
                    ps_cs[0:1, 0:B],
                    ones_col[:],
                    babs[:],
                    start=(kt == 0),
                    stop=(kt == NT - 1),
                )
            cs_sb = const.tile([1, B], f32)
            nc.scalar.copy(cs_sb[:], ps_cs[0:1, 0:B])
            cmax_l = const.tile([1, 1], f32)
            nc.vector.tensor_reduce(
                cmax_l[:], cs_sb[:], axis=mybir.AxisListType.X,
                op=mybir.AluOpType.max,
            )

            rs_d = dram.tile([P, NT], f32)
            rs_do = dram.tile([P, NT], f32, addr_space="Shared")
            cm_d = dram.tile([1, 1], f32)
            cm_do = dram.tile([1, 1], f32, addr_space="Shared")
            nc.sync.dma_start(out=rs_d[:], in_=rs[:])
            nc.sync.dma_start(out=cm_d[:], in_=cmax_l[:])
            nc.gpsimd.collective_compute(
                "AllReduce", mybir.AluOpType.add, replica_groups=rg,
                ins=[rs_d.opt()], outs=[rs_do.opt()],
            )
            nc.gpsimd.collective_compute(
                "AllReduce", mybir.AluOpType.max, replica_groups=rg,
                ins=[cm_d.opt()], outs=[cm_do.opt()],
            )
            rs_full = const.tile([P, NT], f32)
            cmax = const.tile([1, 1], f32)
            nc.sync.dma_start(out=rs_full[:], in_=rs_do[:])
            nc.sync.dma_start(out=cmax[:], in_=cm_do[:])

            rvec = const.tile([P, 1], f32)
            nc.vector.tensor_reduce(
                rvec[:], rs_full[:], axis=mybir.AxisListType.X,
                op=mybir.AluOpType.max,
            )
            ps_t = pssmall.tile([P, 512], f32, tag="small", name="ps_t")
            nc.tensor.transpose(ps_t[0:1, 0:P], rvec[:], ident_f32[:])
            rvec_t = const.tile([1, P], f32)
            nc.scalar.copy(rvec_t[:], ps_t[0:1, 0:P])
            rmax = const.tile([1, 1], f32)
            nc.vector.tensor_reduce(
                rmax[:], rvec_t[:], axis=mybir.AxisListType.X,
                op=mybir.AluOpType.max,
            )

            prod = const.tile([1, 1], f32)
            nc.vector.tensor_tensor(
                out=prod[:], in0=rmax[:], in1=cmax[:], op=mybir.AluOpType.mult
            )
            sq = const.tile([1, 1], f32)
            nc.scalar.sqrt(sq[:], prod[:])
            sval = const.tile([1, 1], f32)
            nc.vector.reciprocal(sval[:], sq[:])
            ps_b = pssmall.tile([P, 512], f32, tag="small", name="ps_b")
            nc.tensor.matmul(
                ps_b[0:P, 0:1], ones_row[:], sval[:], start=True, stop=True
            )
            svec = const.tile([P, 1], f32)
            nc.scalar.copy(svec[:], ps_b[0:P, 0:1])

            for kt in range(NT):
                if use_master:
                    nc.scalar.activation(
                        c_master[:, kt, :], c_master[:, kt, :],
                        mybir.ActivationFunctionType.Copy, scale=svec[:],
                    )
                    nc.vector.tensor_copy(c_mm[:, kt, :], c_master[:, kt, :])
                else:
                    nc.scalar.activation(
                        c_mm[:, kt, :], stage[:, kt, :],
                        mybir.ActivationFunctionType.Copy, scale=svec[:],
                    )

            emit_transposes(0, range(NT))
            emit_gathers(0)

            # ================= iterations =================
            for it in range(ITERS):
                last = it == ITERS - 1

                # phase A: wtwn[:, j*4+nt, :] = -0.5 G[(j,nt)-block, own]
                for nt in range(NBT):
                    for j in range(N_CORES):
                        rt = j * NBT + nt
                        pa = panels.tile([P, NT, P], mmdt, tag="panel",
                                         name="pa")
                        nc.sync.dma_start(
                            out=pa[:],
                            in_=wstc[it][nt][j * P: (j + 1) * P, :].rearrange(
                                "p (kt c) -> p kt c", kt=NT, c=P
                            ),
                        )
                        psg = psmm.tile([P, B], f32, tag="mm", name="psg")
                        for kt in range(NT):
                            nc.tensor.matmul(
                                psg[:],
                                pa[:, kt, :],
                                c_mm[:, kt, :],
                                start=(kt == 0),
                                stop=(kt == NT - 1),
                            )
                        nc.scalar.activation(
                            wtwn[:, rt, :], psg[:],
                            mybir.ActivationFunctionType.Copy, scale=-0.5,
                        )

                # phase B + fused epilogue per row-tile mt
                wT = wstT[it].rearrange(
                    "(j blk) c -> j blk c", j=N_CORES
                )
                for mt in range(NT):
                    pt = panels.tile([P, NT, P], mmdt, tag="panel", name="pt")
                    nc.sync.dma_start(
                        out=pt[:],
                        in_=wT[:, mt * NBT * P: (mt + 1) * NBT * P, :]
                        .rearrange("j (p qt) c -> p j (qt c)", p=P, qt=NBT),
                    )
                    psu = psmm.tile([P, B], f32, tag="mm", name="psu")
                    for g in range(NT):
                        nc.tensor.matmul(
                            psu[:],
                            pt[:, g, :],
                            wtwn[:, g, :],
                            start=(g == 0),
                            stop=(g == NT - 1),
                        )
                    if use_master:
                        nc.vector.scalar_tensor_tensor(
                            out=c_master[:, mt, :],
                            in0=c_master[:, mt, :],
                            scalar=1.5,
                            in1=psu[:],
                            op0=mybir.AluOpType.mult,
                            op1=mybir.AluOpType.add,
                        )
                        nc.vector.tensor_copy(c_mm[:, mt, :], c_master[:, mt, :])
                    else:
                        nc.vector.scalar_tensor_tensor(
                            out=c_mm[:, mt, :],
                            in0=c_mm[:, mt, :],
                            scalar=1.5,
                            in1=psu[:],
                            op0=mybir.AluOpType.mult,
                            op1=mybir.AluOpType.add,
                        )
                    if not last:
                        emit_transposes(it + 1, [mt])

                if not last:
                    emit_gathers(it + 1)

            # ================= output =================
            outr = out.rearrange("(kt p) n -> p kt n", p=P)
            if use_master:
                nc.sync.dma_start(out=outr, in_=c_master[:, :, :])
            else:
                nc.sync.dma_start(out=outr, in_=c_mm.bitcast(f32)[:, :, :])

    nc.compile()
    return nc


_NC_CACHE = {}


def _get_nc():
    key = (ITERS, MM_DTYPE)
    if key not in _NC_CACHE:
        _NC_CACHE[key] = _build()
    return _NC_CACHE[key]


def kernel(weight: np.ndarray, **kwargs) -> np.ndarray:
    assert weight.shape == (D, D) and weight.dtype == np.float32
    nc = _get_nc()
    in_maps = [
        {"wblk": np.ascontiguousarray(weight[:, c * B: (c + 1) * B])}
        for c in range(N_CORES)
    ]
    res = run_bass_kernel_spmd(
        nc, in_maps, core_ids=list(range(N_CORES)),
        trace=bool(int(os.environ.get("BB_TRACE", "0"))),
    )
    full = np.concatenate(
        [res.results[c]["out"] for c in range(N_CORES)], axis=1
    )
    if kwargs.get("return_res"):
        return full, res
    return full


# revision 9
# speedup vs baseline: 1.2967x; 1.1004x over previous
"""Distributed Bjorck-Bowie orthonormalization of a 4096x4096 fp32 matrix
on 8 Trainium2 NeuronCores.

Algorithm (reference):
    s = 1/sqrt(max col abs-sum * max row abs-sum)
    w = W * s
    12x:  G = w^T w ;  w = 1.5 w - 0.5 w G

Distribution: column-sharded. Core i owns C = w[:, 512i:512(i+1)] (fp32
master + matmul-dtype copy in SBUF). Both w and w^T are regathered every
iteration in partition-major tile layouts:
  wst (4 chunks, one per own-col tile nt): chunk = AllGather of
      ag_in[nt*128:(nt+1)*128] where ag_in row (nt*128+p) holds
      [kt, c]-contiguous 8KB spans -> A-panels stream at full DMA width.
  wstT: single AllGather of agT_in, row (mt*512 + p*4 + qt), giving
      1KB-contiguous B-panel lines.
Per core, per iteration:
  phase A: wtwn = -0.5 * G[:, own]; out[r, a] = sum_k w[k,r] C[k,a]
           stationary = A-panels (8 per chunk), moving = c_mm tiles
  phase B: psU = -0.5 * (w G)[:, own]; stationary = B-panels, moving = wtwn
  epilogue (fused): c_master = 1.5*c_master + psU; cast c_mm; PE-transpose
      own tiles into the next agT_in.
AG(wst) is chunked so phase A starts ~38us after the epilogue; AG(wstT)
hides under phase A.

Matmul dtype: bfloat16 (fp32 masters, ~1.4e-3 rel) or float32r
(13-bit mantissa, ~2e-4 rel, 2x bytes).
"""

import os

import numpy as np

import concourse.mybir as mybir
import concourse.tile as tile
from concourse import bacc
from concourse.bass import ts
from concourse.bass_utils import run_bass_kernel_spmd
from concourse.masks import make_identity

N_CORES = 8
D = 4096
B = D // N_CORES        # 512
P = 128
NT = D // P             # 32
NBT = B // P            # 4
ITERS = int(os.environ.get("BB_ITERS", "12"))
MM_DTYPE = os.environ.get("BB_MM_DTYPE", "bfloat16")

f32 = mybir.dt.float32


def _build():
    mmdt = getattr(mybir.dt, MM_DTYPE)
    use_master = MM_DTYPE == "bfloat16"

    nc = bacc.Bacc(
        "TRN2",
        target_bir_lowering=False,
        debug=False,
        num_devices=N_CORES,
    )
    wblk = nc.dram_tensor("wblk", [D, B], f32, kind="ExternalInput").ap()
    out = nc.dram_tensor("out", [D, B], f32, kind="ExternalOutput").ap()

    rg = [list(range(N_CORES))]

    with tile.TileContext(nc) as tc:
        with (
            tc.tile_pool(name="big", bufs=1) as big,
            tc.tile_pool(name="panels", bufs=4) as panels,
            tc.tile_pool(name="work", bufs=3) as work,
            tc.tile_pool(name="const", bufs=1) as const,
            tc.tile_pool(name="psmm", bufs=4, space="PSUM") as psmm,
            tc.tile_pool(name="pssmall", bufs=3, space="PSUM") as pssmall,
            tc.tile_pool(name="dram", bufs=1, space="DRAM") as dram,
        ):
            # ---- persistent state ----
            if use_master:
                c_master = big.tile([P, NT, B], f32)
            c_mm = big.tile([P, NT, B], mmdt)
            wtwn = big.tile([P, NT, B], mmdt)

            ident_mm = const.tile([P, P], mmdt)
            make_identity(nc, ident_mm)
            ident_f32 = const.tile([P, P], f32)
            make_identity(nc, ident_f32)
            ones_col = const.tile([P, 1], mmdt)
            nc.vector.memset(ones_col[:], 1.0)
            ones_row = const.tile([1, P], f32)
            nc.vector.memset(ones_row[:], 1.0)

            # AllGather buffers.
            # ag_in[j]: [512, 4096]; row nt*128+p holds (kt,c) spans of
            #   c_mm[p, :, nt*128:+128]  (8KB contiguous per row)
            # wstc[j][nt]: AG out [8*128, 4096] (rank-stacked chunk)
            # agT_in[j]: [16384, 128]; row mt*512 + p*4 + qt = transposed
            #   tile lines; wstT[j]: AG out [8*16384, 128]
            ag_in = [
                dram.tile([NBT * P, NT * P], mmdt, name=f"ag_in{j}")
                for j in range(ITERS)
            ]
            wstc = [
                [
                    dram.tile([N_CORES * P, NT * P], mmdt,
                              addr_space="Shared", name=f"wstc{j}_{nt}")
                    for nt in range(NBT)
                ]
                for j in range(ITERS)
            ]
            agT_in = [
                dram.tile([NT * NBT * P, P], mmdt, name=f"agT_in{j}")
                for j in range(ITERS)
            ]
            wstTc = [
                [
                    dram.tile([N_CORES * (NT // 4) * NBT * P, P], mmdt,
                              addr_space="Shared", name=f"wstTc{j}_{tq}")
                    for tq in range(4)
                ]
                for j in range(ITERS)
            ]
            TCH = (NT // 4) * NBT * P  # rows per agT_in chunk (4096)

            def emit_ag_c(j):
                for nt in range(NBT):
                    agr = ag_in[j][nt * P: (nt + 1) * P, :].rearrange(
                        "p (kt c) -> p kt c", kt=NT, c=P
                    )
                    nc.scalar.dma_start(out=agr, in_=c_mm[:, :, ts(nt, P)])
                for nt in range(NBT):
                    nc.gpsimd.collective_compute(
                        "AllGather", mybir.AluOpType.bypass, replica_groups=rg,
                        ins=[ag_in[j][nt * P: (nt + 1) * P, :].opt()],
                        outs=[wstc[j][nt].opt()],
                    )

            def emit_ag_T(j, tq):
                nc.gpsimd.collective_compute(
                    "AllGather", mybir.AluOpType.bypass, replica_groups=rg,
                    ins=[agT_in[j][tq * TCH: (tq + 1) * TCH, :].opt()],
                    outs=[wstTc[j][tq].opt()],
                )

            def emit_transposes(j, mt_range):
                """Own-block transposed tiles -> agT_in[j] rows mt*512+p*4+qt."""
                for mt in mt_range:
                    pstm = pssmall.tile([P, 512], mmdt, tag="small", name="pstm")
                    for qt in range(NBT):
                        nc.tensor.transpose(
                            pstm[:, ts(qt, P)], c_mm[:, mt, ts(qt, P)],
                            ident_mm[:],
                        )
                    stg = work.tile([P, NBT * P], mmdt, name="stg")
                    nc.scalar.copy(stg[:], pstm[:])
                    o = agT_in[j][mt * NBT * P: (mt + 1) * NBT * P, :]
                    nc.gpsimd.dma_start(
                        out=o.rearrange("(p qt) c -> p qt c", p=P, qt=NBT),
                        in_=stg.rearrange("p (qt c) -> p qt c", qt=NBT),
                    )

            # ================= preamble: load + scale =================
            if use_master:
                stage = c_master
            else:
                stage = big.tile([P, NT, B], f32, name="stage")
            for kt in range(NT):
                nc.sync.dma_start(out=stage[:, kt, :], in_=wblk[ts(kt, P), :])

            rs = const.tile([P, NT], f32)
            for kt in range(NT):
                nc.vector.tensor_reduce(
                    rs[:, kt: kt + 1],
                    stage[:, kt, :],
                    axis=mybir.AxisListType.X,
                    op=mybir.AluOpType.add,
                    apply_absolute_value=True,
                )
            ps_cs = pssmall.tile([P, 512], f32, tag="small", name="ps_cs")
            for kt in range(NT):
                babs = work.tile([P, B], mmdt, name="babs")
                nc.scalar.activation(
                    babs[:], stage[:, kt, :], mybir.ActivationFunctionType.Abs
                )
                nc.tensor.matmul(
                    ps_cs[0:1, 0:B],
                    ones_col[:],
                    babs[:],
                    start=(kt == 0),
                    stop=(kt == NT - 1),
                )
            cs_sb = const.tile([1, B], f32)
            nc.scalar.copy(cs_sb[:], ps_cs[0:1, 0:B])
            cmax_l = const.tile([1, 1], f32)
            nc.vector.tensor_reduce(
                cmax_l[:], cs_sb[:], axis=mybir.AxisListType.X,
                op=mybir.AluOpType.max,
            )

            rs_d = dram.tile([P, NT], f32)
            rs_do = dram.tile([P, NT], f32, addr_space="Shared")
            cm_d = dram.tile([1, 1], f32)
            cm_do = dram.tile([1, 1], f32, addr_space="Shared")
            nc.sync.dma_start(out=rs_d[:], in_=rs[:])
            nc.sync.dma_start(out=cm_d[:], in_=cmax_l[:])
            nc.gpsimd.collective_compute(
                "AllReduce", mybir.AluOpType.add, replica_groups=rg,
                ins=[rs_d.opt()], outs=[rs_do.opt()],
            )
            nc.gpsimd.collective_compute(
                "AllReduce", mybir.AluOpType.max, replica_groups=rg,
                ins=[cm_d.opt()], outs=[cm_do.opt()],
            )
            rs_full = const.tile([P, NT], f32)
            cmax = const.tile([1, 1], f32)
            nc.sync.dma_start(out=rs_full[:], in_=rs_do[:])
            nc.sync.dma_start(out=cmax[:], in_=cm_do[:])

            rvec = const.tile([P, 1], f32)
            nc.vector.tensor_reduce(
                rvec[:], rs_full[:], axis=mybir.AxisListType.X,
                op=mybir.AluOpType.max,
            )
            ps_t = pssmall.tile([P, 512], f32, tag="small", name="ps_t")
            nc.tensor.transpose(ps_t[0:1, 0:P], rvec[:], ident_f32[:])
            rvec_t = const.tile([1, P], f32)
            nc.scalar.copy(rvec_t[:], ps_t[0:1, 0:P])
            rmax = const.tile([1, 1], f32)
            nc.vector.tensor_reduce(
                rmax[:], rvec_t[:], axis=mybir.AxisListType.X,
                op=mybir.AluOpType.max,
            )

            prod = const.tile([1, 1], f32)
            nc.vector.tensor_tensor(
                out=prod[:], in0=rmax[:], in1=cmax[:], op=mybir.AluOpType.mult
            )
            sq = const.tile([1, 1], f32)
            nc.scalar.sqrt(sq[:], prod[:])
            sval = const.tile([1, 1], f32)
            nc.vector.reciprocal(sval[:], sq[:])
            ps_b = pssmall.tile([P, 512], f32, tag="small", name="ps_b")
            nc.tensor.matmul(
                ps_b[0:P, 0:1], ones_row[:], sval[:], start=True, stop=True
            )
            svec = const.tile([P, 1], f32)
            nc.scalar.copy(svec[:], ps_b[0:P, 0:1])

            for kt in range(NT):
                if use_master:
                    nc.scalar.activation(
                        c_master[:, kt, :], c_master[:, kt, :],
                        mybir.ActivationFunctionType.Copy, scale=svec[:],
                    )
                    nc.vector.tensor_copy(c_mm[:, kt, :], c_master[:, kt, :])
                else:
                    nc.scalar.activation(
                        c_mm[:, kt, :], stage[:, kt, :],
                        mybir.ActivationFunctionType.Copy, scale=svec[:],
                    )

            emit_transposes(0, range(NT))
            emit_ag_c(0)
            for tq in range(4):
                emit_ag_T(0, tq)

            # ================= iterations =================
            for it in range(ITERS):
                last = it == ITERS - 1

                # phase A: wtwn[:, j*4+nt, :] = -0.5 G[(j,nt)-block, own]
                for nt in range(NBT):
                    for j in range(N_CORES):
                        rt = j * NBT + nt
                        pa = panels.tile([P, NT, P], mmdt, tag="panel",
                                         name="pa")
                        nc.sync.dma_start(
                            out=pa[:],
                            in_=wstc[it][nt][j * P: (j + 1) * P, :].rearrange(
                                "p (kt c) -> p kt c", kt=NT, c=P
                            ),
                        )
                        psg = psmm.tile([P, B], f32, tag="mm", name="psg")
                        for kt in range(NT):
                            nc.tensor.matmul(
                                psg[:],
                                pa[:, kt, :],
                                c_mm[:, kt, :],
                                start=(kt == 0),
                                stop=(kt == NT - 1),
                            )
                        nc.scalar.activation(
                            wtwn[:, rt, :], psg[:],
                            mybir.ActivationFunctionType.Copy, scale=-0.5,
                        )

                # phase B + fused epilogue per row-tile mt
                for mt in range(NT):
                    tq, mtl = mt // 8, mt % 8
                    wT = wstTc[it][tq].rearrange(
                        "(j blk) c -> j blk c", j=N_CORES
                    )
                    pt = panels.tile([P, NT, P], mmdt, tag="panel", name="pt")
                    nc.sync.dma_start(
                        out=pt[:],
                        in_=wT[:, mtl * NBT * P: (mtl + 1) * NBT * P, :]
                        .rearrange("j (p qt) c -> p j (qt c)", p=P, qt=NBT),
                    )
                    psu = psmm.tile([P, B], f32, tag="mm", name="psu")
                    for g in range(NT):
                        nc.tensor.matmul(
                            psu[:],
                            pt[:, g, :],
                            wtwn[:, g, :],
                            start=(g == 0),
                            stop=(g == NT - 1),
                        )
                    if use_master:
                        nc.vector.scalar_tensor_tensor(
                            out=c_master[:, mt, :],
                            in0=c_master[:, mt, :],
                            scalar=1.5,
                            in1=psu[:],
                            op0=mybir.AluOpType.mult,
                            op1=mybir.AluOpType.add,
                        )
                        nc.vector.tensor_copy(c_mm[:, mt, :], c_master[:, mt, :])
                    else:
                        nc.vector.scalar_tensor_tensor(
                            out=c_mm[:, mt, :],
                            in0=c_mm[:, mt, :],
                            scalar=1.5,
                            in1=psu[:],
                            op0=mybir.AluOpType.mult,
                            op1=mybir.AluOpType.add,
                        )
                    if not last:
                        emit_transposes(it + 1, [mt])
                        if mt == 7:
                            emit_ag_T(it + 1, 0)
                        elif mt == 15:
                            emit_ag_T(it + 1, 1)

                if not last:
                    emit_ag_c(it + 1)
                    emit_ag_T(it + 1, 2)
                    emit_ag_T(it + 1, 3)

            # ================= output =================
            outr = out.rearrange("(kt p) n -> p kt n", p=P)
            if use_master:
                nc.sync.dma_start(out=outr, in_=c_master[:, :, :])
            else:
                nc.sync.dma_start(out=outr, in_=c_mm.bitcast(f32)[:, :, :])

    nc.compile()
    return nc


_NC_CACHE = {}


def _get_nc():
    key = (ITERS, MM_DTYPE)
    if key not in _NC_CACHE:
        _NC_CACHE[key] = _build()
    return _NC_CACHE[key]


def kernel(weight: np.ndarray, **kwargs) -> np.ndarray:
    assert weight.shape == (D, D) and weight.dtype == np.float32
    nc = _get_nc()
    in_maps = [
        {"wblk": np.ascontiguousarray(weight[:, c * B: (c + 1) * B])}
        for c in range(N_CORES)
    ]
    res = run_bass_kernel_spmd(
        nc, in_maps, core_ids=list(range(N_CORES)),
        trace=bool(int(os.environ.get("BB_TRACE", "0"))),
    )
    full = np.concatenate(
        [res.results[c]["out"] for c in range(N_CORES)], axis=1
    )
    if kwargs.get("return_res"):
        return full, res
    return full
